# revision 1
# baseline (speedup 1.0000x reference)
"""Trainium2 Bass kernel for nn_CausalSelfAttention_28467043237962.

Sharding: 8 cores = 2 batches x 4 head-groups (4 heads / 256 dims each).
Per core: fused QKV projection (bf16 matmuls, fp32 PSUM accumulation),
per-head RMS-norm (fp32 stats) + RoPE (bf16 elementwise, only the 16 active
frequencies), causal attention in transposed-scores layout (scores_T[s,t], so
softmax needs no cross-partition reduction: denominators come from a
[v|ones64] column block in the AV matmul and divide out after a packed
reciprocal + DRAM-broadcast), output projection, host-side sum of the 4
per-batch partials. Mask blocks are classified on the host (skip/pass/
tri/general) so causal masking skips half the compute and works for any
additive mask, including all-zeros.

Self-contained: builds one SPMD Bass program and runs it on cores 0-7 via
concourse.bass_utils.run_bass_kernel_spmd.
"""
import sys

sys.path.insert(0, "/opt/trn_rl_repo")

from contextlib import ExitStack

import numpy as np
import ml_dtypes

import concourse.bass as bass
import concourse.tile as tile
import concourse.mybir as mybir
from concourse.vector_clock import ScopedClock
from concourse.bass_utils import run_bass_kernel_spmd

F32 = mybir.dt.float32
F32R = mybir.dt.float32r
BF16 = mybir.dt.bfloat16

B, T, DIM = 2, 2048, 1024
H, HD = 16, 64
SCALE = 0.12
ROPE_BASE = 1024.0
EPS = 1e-6
G = 4          # head-groups = cores per batch
HPG = 4        # heads per group
GD = HPG * HD  # 256 dims per group
NT = T // 128  # 16 s/t tiles of 128
NJ = T // 512  # 4 t-blocks of 512

TRACE = False          # set by test.py for profiling runs
LAST_EXEC_NS = None    # filled when TRACE


class _TileContextFixed(tile.TileContext):
    """Workaround for this container's walrus build: the kernel-tail InstDrain
    may carry only one sync wait. Spread the tail waits over single-wait NOPs
    on the sync engine before a wait-free drain."""

    def _drain_and_barrier(self, tick_clock, wait_clock):
        nc = self.nc
        collector = nc.sync.nop(nofuse=True, hint="tail_wait_collector")
        wait_clock.add_sem_waits(
            collector.ins, ScopedClock({None: tick_clock.global_clock})
        )
        si = collector.ins.sync_info
        waits = list(si.on_wait or [])
        if len(waits) > 1:
            si.on_wait = waits[:1]
            for w in waits[1:]:
                extra = nc.sync.nop(nofuse=True, hint="tail_wait")
                esi = extra.ins.sync_info
                if esi is None:
                    extra.ins.sync_info = mybir.SyncInfo(on_wait=[w], on_update=[])
                else:
                    esi.on_wait = [w]
        nc.sync.drain()
        nc.all_engine_barrier()
        assert self.sems is not None
        popped = nc._tile_sem_poison_stack.pop()
        assert popped is self._sem_poison
        nc.clear_and_free_semaphores(list(self.sems.allocated().values()))
        nc.all_engine_barrier()


def _split_excess_waits(nc, max_waits=1):
    """This container's walrus build rejects instructions carrying more than
    one embedded sync wait. Move excess waits onto dedicated NOPs inserted
    just before the instruction on the same engine."""
    ctr = [0]
    for func in nc.m.functions:
        for block in func.blocks:
            out = []
            for inst in block.instructions:
                si = inst.sync_info
                waits = list(si.on_wait) if si and si.on_wait else []
                limit = 0 if isinstance(inst, mybir.InstDrain) else max_waits
                if len(waits) > limit:
                    keep = waits[:limit]
                    extra = waits[limit:]
                    for w in extra:
                        ctr[0] += 1
                        nop = mybir.InstNoOp(
                            name=f"waitnop-{ctr[0]}",
                            sync_info=mybir.SyncInfo(on_wait=[w], on_update=[]),
                            bass_nofuse=True,
                            engine=inst.engine,
                        )
                        out.append(nop)
                    si.on_wait = keep
                out.append(inst)
            block.instructions = out


def _rope_tables():
    keep = HD // 4  # 16 active frequencies; dims 16:32 of each half are identity
    active = (1.0 / ROPE_BASE) ** np.linspace(0.0, 1.0, keep, dtype=np.float32)
    th = np.arange(T, dtype=np.float32)[:, None] * active[None, :]
    return np.cos(th).astype(np.float32), np.sin(th).astype(np.float32)


def _classify_blocks(mask):
    """mask [T,T] additive, indexed (t, s). Block = (s-tile i of 128) x
    (t-block jb of 512). Returns cls[i][jb] in {skip, pass, tri, gen}."""
    cls = []
    for i in range(NT):
        row = []
        for jb in range(NJ):
            blk = mask[512 * jb:512 * (jb + 1), 128 * i:128 * (i + 1)]
            big_neg = blk <= -1e8
            zero = blk == 0.0
            if big_neg.all():
                row.append("skip")
            elif zero.all():
                row.append("pass")
            elif (big_neg | zero).all():
                tt = np.arange(512 * jb, 512 * (jb + 1))[:, None]
                ss = np.arange(128 * i, 128 * (i + 1))[None, :]
                row.append("tri" if np.array_equal(zero, tt >= ss) else "gen")
            else:
                row.append("gen")
        cls.append(row)
    return cls


def _build_program(cls, has_gen):
    nc = bass.Bass()
    xT = nc.declare_dram_parameter("xT", [DIM, T], BF16, isOutput=False)
    wq = nc.declare_dram_parameter("wqkvT", [DIM, 3 * GD], BF16, isOutput=False)
    wo = nc.declare_dram_parameter("woT", [GD, DIM], BF16, isOutput=False)
    ve = nc.declare_dram_parameter("ve_s", [T, GD], BF16, isOutput=False)
    lam = nc.declare_dram_parameter("lam", [128, 2], F32, isOutput=False)
    rC = nc.declare_dram_parameter("ropeC", [T, 16], BF16, isOutput=False)
    rS = nc.declare_dram_parameter("ropeS", [T, 16], BF16, isOutput=False)
    tri = nc.declare_dram_parameter("tri01", [4, 128, 512], BF16, isOutput=False)
    md = None
    if has_gen:
        md = nc.declare_dram_parameter("maskdiv", [T, T], F32, isOutput=False)
    outT = nc.declare_dram_parameter("outT", [DIM, T], F32, isOutput=True)
    rec_dram = nc.dram_tensor("rec_scratch", [HPG, T], F32)

    # per-(i, jb): first/last valid s-tile for AV accumulation start/stop
    first_i = [None] * NJ
    last_i = [None] * NJ
    for jb in range(NJ):
        valid = [i for i in range(NT) if cls[i][jb] != "skip"]
        if valid:
            first_i[jb] = valid[0]
            last_i[jb] = valid[-1]

    with _TileContextFixed(nc) as tc, ExitStack() as ctx:
        S = ctx.enter_context(tc.tile_pool(name="singles", bufs=1))

        wq_sb = S.tile([128, 8, 3 * GD], BF16, tag="wq_sb")
        for ds in range(8):
            nc.sync.dma_start(out=wq_sb[:, ds, :],
                              in_=wq[128 * ds:128 * (ds + 1), :])
        wo_sb = S.tile([128, 2, DIM], BF16, tag="wo_sb")
        for dc in range(2):
            nc.sync.dma_start(out=wo_sb[:, dc, :],
                              in_=wo[128 * dc:128 * (dc + 1), :])
        lam_sb = S.tile([128, 2], F32, tag="lam_sb")
        nc.sync.dma_start(out=lam_sb, in_=lam[:, :])
        rc_sb = S.tile([128, NT, 16], BF16, tag="rc_sb")
        nc.sync.dma_start(out=rc_sb, in_=rC.rearrange("(s p) k -> p s k", p=128))
        rs_sb = S.tile([128, NT, 16], BF16, tag="rs_sb")
        nc.sync.dma_start(out=rs_sb, in_=rS.rearrange("(s p) k -> p s k", p=128))
        tri_sb = S.tile([128, 4, 512], BF16, tag="tri_sb")
        nc.sync.dma_start(out=tri_sb, in_=tri.rearrange("m p c -> p m c"))

        eps_sb = S.tile([128, 1], F32, tag="eps_sb")
        nc.vector.memset(eps_sb, EPS)
        ident = S.tile([128, 128], BF16, tag="ident")
        from concourse.masks import make_identity
        make_identity(nc, ident)

        # v_aug[p, tt, h, 0:128]: even h -> [v | ones], odd h -> [ones | v]
        v_aug = S.tile([128, NT, HPG, 128], BF16, tag="v_aug")
        v5 = v_aug.rearrange("p t (a b) c -> p t a b c", b=2)
        nc.vector.memset(v5[:, :, :, 0, 64:128], 1.0)
        nc.vector.memset(v5[:, :, :, 1, 0:64], 1.0)

        qT = [S.tile([128, T], BF16, tag=f"qT{p}", name=f"qT{p}") for p in range(2)]
        kT = [S.tile([128, T], BF16, tag=f"kT{p}", name=f"kT{p}") for p in range(2)]
        yT = [S.tile([128, T], BF16, tag=f"yT{p}", name=f"yT{p}") for p in range(2)]

        # ---------------- stage A: QKV + norm + rope + transpose -------------
        with tc.tile_pool(name="a_sb", bufs=3) as A, \
             tc.tile_pool(name="a_ps", bufs=2, space="PSUM") as QP, \
             tc.tile_pool(name="tr_ps", bufs=4, space="PSUM") as TP:
            for tt in range(NT):
                xt = A.tile([128, 8, 128], BF16, tag="xt")
                nc.sync.dma_start(
                    out=xt,
                    in_=xT[:, 128 * tt:128 * (tt + 1)].rearrange(
                        "(s p) t -> p s t", p=128),
                )
                qp = QP.tile([128, 3 * GD], F32, tag="qp")
                for ds in range(8):
                    lhsT = xt[:, ds, :]
                    nc.tensor.matmul(
                        qp[:, 0:512], lhsT, wq_sb[:, ds, 0:512],
                        start=(ds == 0), stop=(ds == 7))
                    nc.tensor.matmul(
                        qp[:, 512:768], lhsT, wq_sb[:, ds, 512:768],
                        start=(ds == 0), stop=(ds == 7))

                sq = A.tile([128, 3 * GD], F32, tag="sq")
                nc.scalar.activation(sq, qp, mybir.ActivationFunctionType.Square)
                ms = A.tile([128, 12], F32, tag="ms")
                nc.vector.tensor_reduce(
                    ms, sq.rearrange("p (g d) -> p g d", d=HD),
                    axis=mybir.AxisListType.X, op=mybir.AluOpType.add)
                srt = A.tile([128, 12], F32, tag="srt")
                nc.scalar.activation(
                    srt, ms, mybir.ActivationFunctionType.Sqrt,
                    bias=eps_sb[:, 0:1], scale=1.0 / HD)
                rstd = A.tile([128, 12], F32, tag="rstd")
                nc.vector.reciprocal(rstd, srt)
                rstd_bf = A.tile([128, 12], BF16, tag="rstd_bf")
                nc.scalar.copy(rstd_bf, rstd)

                qkv_bf = A.tile([128, 3 * GD], BF16, tag="qkv_bf")
                nc.scalar.copy(qkv_bf, qp)

                qkb = A.tile([128, 2 * GD], BF16, tag="qkb")
                nc.vector.tensor_tensor(
                    qkb[:, 0:GD].rearrange("p (g d) -> p g d", d=HD),
                    qkv_bf[:, 0:GD].rearrange("p (g d) -> p g d", d=HD),
                    rstd_bf[:, 0:4, None].to_broadcast([128, 4, HD]),
                    mybir.AluOpType.mult)
                nc.vector.tensor_tensor(
                    qkb[:, GD:2 * GD].rearrange("p (g d) -> p g d", d=HD),
                    qkv_bf[:, GD:2 * GD].rearrange("p (g d) -> p g d", d=HD),
                    rstd_bf[:, 4:8, None].to_broadcast([128, 4, HD]),
                    mybir.AluOpType.mult)

                vt = A.tile([128, GD], BF16, tag="vt")
                nc.vector.tensor_tensor(
                    vt.rearrange("p (g d) -> p g d", d=HD),
                    qkv_bf[:, 2 * GD:3 * GD].rearrange("p (g d) -> p g d", d=HD),
                    rstd_bf[:, 8:12, None].to_broadcast([128, 4, HD]),
                    mybir.AluOpType.mult)

                ve_sb = A.tile([128, GD], BF16, tag="ve_sb")
                nc.sync.dma_start(out=ve_sb, in_=ve[128 * tt:128 * (tt + 1), :])
                vl = A.tile([128, GD], BF16, tag="vl")
                nc.vector.tensor_scalar_mul(vl, ve_sb, lam_sb[:, 1:2])

                # blend into v_aug (even heads -> cols 0:64, odd -> 64:128)
                vt4 = vt.rearrange("p (a b d) -> p a b d", a=2, d=HD)
                vl4 = vl.rearrange("p (a b d) -> p a b d", a=2, d=HD)
                nc.vector.scalar_tensor_tensor(
                    out=v5[:, tt, :, 0, 0:64], in0=vt4[:, :, 0, :],
                    scalar=lam_sb[:, 0:1], in1=vl4[:, :, 0, :],
                    op0=mybir.AluOpType.mult, op1=mybir.AluOpType.add)
                nc.vector.scalar_tensor_tensor(
                    out=v5[:, tt, :, 1, 64:128], in0=vt4[:, :, 1, :],
                    scalar=lam_sb[:, 0:1], in1=vl4[:, :, 1, :],
                    op0=mybir.AluOpType.mult, op1=mybir.AluOpType.add)

                # rope (active dims 0:16 and 32:48 of each head)
                v6 = qkb.rearrange("p (s g d) -> p s g d", s=2, d=HD)
                x0 = v6[:, :, :, 0:16]
                x32 = v6[:, :, :, 32:48]
                cb = rc_sb[:, tt, None, None, :].to_broadcast([128, 2, HPG, 16])
                sb = rs_sb[:, tt, None, None, :].to_broadcast([128, 2, HPG, 16])
                ra = A.tile([128, 2, HPG, 16], BF16, tag="ra")
                rb = A.tile([128, 2, HPG, 16], BF16, tag="rb")
                nc.vector.tensor_tensor(ra, x0, sb, mybir.AluOpType.mult)
                nc.vector.tensor_tensor(rb, x32, sb, mybir.AluOpType.mult)
                nc.vector.tensor_tensor(x0, x0, cb, mybir.AluOpType.mult)
                nc.vector.tensor_tensor(x32, x32, cb, mybir.AluOpType.mult)
                nc.vector.tensor_tensor(x0, x0, rb, mybir.AluOpType.add)
                nc.vector.tensor_tensor(x32, x32, ra, mybir.AluOpType.subtract)

                # transpose q,k head-pairs to [hd, t]
                for ec in range(4):
                    tp = TP.tile([128, 128], BF16, tag="tp")
                    nc.tensor.transpose(
                        tp, qkb[:, 128 * ec:128 * (ec + 1)], ident)
                    dst = (qT if ec < 2 else kT)[ec % 2]
                    if ec % 2 == 0:
                        nc.scalar.copy(dst[:, 128 * tt:128 * (tt + 1)], tp)
                    else:
                        nc.vector.tensor_copy(
                            out=dst[:, 128 * tt:128 * (tt + 1)], in_=tp)

        # ---------------- stage B: attention, (head, t-half) outer ----------
        with tc.tile_pool(name="sc_ps", bufs=2, space="PSUM") as SC, \
             tc.tile_pool(name="av_ps", bufs=2, space="PSUM") as AV, \
             tc.tile_pool(name="ex_sb", bufs=4) as EX, \
             tc.tile_pool(name="nrm_sb", bufs=2) as NR, \
             tc.tile_pool(name="md_sb", bufs=2) as MD:
            for h in range(HPG):
                par, pair = h % 2, h // 2
                rlo, rhi = 64 * par, 64 * par + 64
                drow = 64 if par == 0 else 0
                for jj in range(2):
                    av = AV.tile([128, 1024], F32, tag="av")
                    for i in range(NT):
                        jbs = [jb for jb in (2 * jj, 2 * jj + 1)
                               if cls[i][jb] != "skip"]
                        if not jbs:
                            continue
                        sc = SC.tile([128, 1024], F32, tag="sc")
                        for jb in jbs:
                            off = 512 * (jb - 2 * jj)
                            nc.tensor.matmul(
                                sc[:, off:off + 512],
                                kT[pair][rlo:rhi, 128 * i:128 * (i + 1)],
                                qT[pair][rlo:rhi, 512 * jb:512 * (jb + 1)],
                                start=True, stop=True)
                            if cls[i][jb] == "gen":
                                mdt = MD.tile([128, 512], F32, tag="mdt")
                                nc.sync.dma_start(
                                    out=mdt,
                                    in_=md[512 * jb:512 * (jb + 1),
                                           128 * i:128 * (i + 1)].rearrange(
                                               "t s -> s t"))
                                nc.vector.tensor_tensor(
                                    sc[:, off:off + 512], sc[:, off:off + 512],
                                    mdt, mybir.AluOpType.add)
                        ex = EX.tile([128, 1024], BF16, tag="ex")
                        lo = 512 * (min(jbs) - 2 * jj)
                        hi = 512 * (max(jbs) - 2 * jj) + 512
                        nc.scalar.activation(
                            ex[:, lo:hi], sc[:, lo:hi],
                            mybir.ActivationFunctionType.Exp, scale=SCALE)
                        for jb in jbs:
                            off = 512 * (jb - 2 * jj)
                            if cls[i][jb] == "tri":
                                nc.vector.tensor_tensor(
                                    ex[:, off:off + 512],
                                    ex[:, off:off + 512],
                                    tri_sb[:, i % 4, :],
                                    mybir.AluOpType.mult)
                            nc.tensor.matmul(
                                av[:, off:off + 512],
                                v_aug[:, i, h, :],
                                ex[:, off:off + 512],
                                start=(first_i[jb] == i),
                                stop=(last_i[jb] == i))
                    # empty column-blocks (pathological masks): den=1, y=0
                    for jb in (2 * jj, 2 * jj + 1):
                        if first_i[jb] is None:
                            off = 512 * (jb - 2 * jj)
                            nc.vector.memset(av[:, off:off + 512], 1.0)
                    # normalize this half: av rows = [y | den-rep] (even h)
                    av_sb = NR.tile([128, 1024], F32, tag="av_sb")
                    nc.vector.tensor_copy(out=av_sb, in_=av)
                    den_pk = NR.tile([128, 8], F32, tag="den_pk")
                    nc.sync.dma_start(
                        out=den_pk, in_=av_sb[drow:drow + 1, :])
                    rec_pk = NR.tile([128, 8], F32, tag="rec_pk")
                    nc.vector.reciprocal(rec_pk, den_pk)
                    hsl = slice(1024 * jj, 1024 * (jj + 1))
                    nc.sync.dma_start(out=rec_dram[h, hsl], in_=rec_pk)
                    rrow_ap = rec_dram[h, hsl]
                    rec_bc = bass.AP(
                        tensor=rrow_ap.tensor,
                        offset=rrow_ap.offset,
                        ap=[[0, 64]] + [list(p) for p in rrow_ap.ap])
                    rout = NR.tile([128, 1024], F32, tag="rout")
                    nc.sync.dma_start(out=rout[rlo:rhi, :], in_=rec_bc)
                    nc.vector.tensor_tensor(
                        yT[pair][rlo:rhi, hsl], av_sb[rlo:rhi, :],
                        rout[rlo:rhi, :], mybir.AluOpType.mult)

        # ---------------- stage C: output projection -------------------------
        with tc.tile_pool(name="po_ps", bufs=4, space="PSUM") as PO, \
             tc.tile_pool(name="os_sb", bufs=4) as OS:
            for ec in range(8):
                for tb in range(NJ):
                    po = PO.tile([128, 512], F32, tag="po")
                    for dc in range(2):
                        nc.tensor.matmul(
                            po,
                            wo_sb[:, dc, 128 * ec:128 * (ec + 1)],
                            yT[dc][:, 512 * tb:512 * (tb + 1)],
                            start=(dc == 0), stop=(dc == 1))
                    os = OS.tile([128, 512], F32, tag="os")
                    if (ec + tb) % 2 == 0:
                        nc.scalar.copy(os, po)
                    else:
                        nc.vector.tensor_copy(out=os, in_=po)
                    nc.sync.dma_start(
                        out=outT[128 * ec:128 * (ec + 1),
                                 512 * tb:512 * (tb + 1)],
                        in_=os)
    _split_excess_waits(nc)
    return nc


def kernel(x, ve, sa_lambdas, attn_mask, qkvo_w):
    global LAST_EXEC_NS
    x = np.ascontiguousarray(np.asarray(x, np.float32))
    ve = np.ascontiguousarray(np.asarray(ve, np.float32))
    sa_lambdas = np.asarray(sa_lambdas, np.float32)
    attn_mask = np.asarray(attn_mask, np.float32)
    qkvo_w = np.asarray(qkvo_w, np.float32)

    ropeC, ropeS = _rope_tables()
    mask = attn_mask[0, 0]
    cls = _classify_blocks(mask)
    has_gen = any(c == "gen" for row in cls for c in row)

    nc = _build_program(cls, has_gen)

    iota = np.arange(512)
    part = np.arange(128)
    diff = part[:, None] - iota[None, :]                               # p - c
    tri01 = np.stack([(diff <= -128 * m) for m in range(4)], 0)
    tri01 = tri01.astype(ml_dtypes.bfloat16)
    lam = np.broadcast_to(sa_lambdas[None, :], (128, 2)).copy()
    rC = ropeC.astype(ml_dtypes.bfloat16)
    rS = ropeS.astype(ml_dtypes.bfloat16)
    maskdiv = (mask / SCALE).astype(np.float32) if has_gen else None

    in_maps = []
    for c in range(8):
        b, g = c // G, c % G
        sl = slice(GD * g, GD * (g + 1))
        wqkvT = np.ascontiguousarray(
            np.concatenate([qkvo_w[k][sl, :] for k in range(3)], 0).T)
        m = {
            "xT": np.ascontiguousarray(x[b].T).astype(ml_dtypes.bfloat16),
            "wqkvT": wqkvT.astype(ml_dtypes.bfloat16),
            "woT": np.ascontiguousarray(
                qkvo_w[3][:, sl].T).astype(ml_dtypes.bfloat16),
            "ve_s": np.ascontiguousarray(
                ve[b][:, sl]).astype(ml_dtypes.bfloat16),
            "lam": lam,
            "ropeC": rC,
            "ropeS": rS,
            "tri01": tri01,
        }
        if has_gen:
            m["maskdiv"] = maskdiv
        in_maps.append(m)

    res = run_bass_kernel_spmd(nc, in_maps, core_ids=list(range(8)),
                               trace=TRACE)
    if TRACE:
        LAST_EXEC_NS = res.exec_time_ns

    out = np.zeros((B, T, DIM), np.float32)
    for c in range(8):
        out[c // G] += res.results[c]["outT"].T
    return out



# revision 10
# speedup vs baseline: 1.1026x; 1.1026x over previous
"""Trainium2 Bass kernel for nn_CausalSelfAttention_28467043237962.

Sharding: 8 cores = 2 batches x 4 head-groups (4 heads / 256 dims each).
Per core: fused QKV projection (bf16 matmuls, fp32 PSUM), per-head RMS-norm
(rstd = exp(-0.5*ln(ms)) so one ACT table set serves norm AND softmax),
RoPE on the 16 active frequencies, causal attention in transposed-scores
layout (scores_T[s,t]; denominators from [v|ones] columns in the AV matmul,
normalized by a direct per-partition reciprocal + one SBUF->SBUF DMA to
shift partitions). Diagonal 128x512 blocks are column-trimmed to the valid
range with a single upper-tri 128x128 mask. Stage emission is interleaved
(QKV tiles 8-15 run while jj=0 softmax exps drain) to keep the PE dense and
HAM-warm. Output projection emits bf16 partials; host sums the 4 groups.

Self-contained: builds one SPMD Bass program and runs it on cores 0-7 via
concourse.bass_utils.run_bass_kernel_spmd.
"""
import sys

sys.path.insert(0, "/opt/trn_rl_repo")

from contextlib import ExitStack

import numpy as np
import ml_dtypes

import concourse.bass as bass
import concourse.tile as tile
import concourse.mybir as mybir
from concourse.vector_clock import ScopedClock
from concourse.bass_utils import run_bass_kernel_spmd

F32 = mybir.dt.float32
BF16 = mybir.dt.bfloat16

B, T, DIM = 2, 2048, 1024
H, HD = 16, 64
SCALE = 0.12
ROPE_BASE = 1024.0
EPS = 1e-6
G = 4          # head-groups = cores per batch
HPG = 4        # heads per group
GD = HPG * HD  # 256 dims per group
NT = T // 128  # 16 s/t tiles of 128
NJ = T // 512  # 4 t-blocks of 512

TRACE = False          # set by test.py for profiling runs
LAST_EXEC_NS = None    # filled when TRACE

TRIM_DIAG = True       # column-trim diagonal blocks to the causal range
GPSIMD_TRI = True      # run the tri-square masks on the GpSimd engine


class _TileContextFixed(tile.TileContext):
    """Workaround for this container's walrus build: the kernel-tail InstDrain
    may carry only one sync wait. Spread the tail waits over single-wait NOPs
    on the sync engine before a wait-free drain."""

    def _drain_and_barrier(self, tick_clock, wait_clock):
        nc = self.nc
        collector = nc.sync.nop(nofuse=True, hint="tail_wait_collector")
        wait_clock.add_sem_waits(
            collector.ins, ScopedClock({None: tick_clock.global_clock})
        )
        si = collector.ins.sync_info
        waits = list(si.on_wait or [])
        if len(waits) > 1:
            si.on_wait = waits[:1]
            for w in waits[1:]:
                extra = nc.sync.nop(nofuse=True, hint="tail_wait")
                esi = extra.ins.sync_info
                if esi is None:
                    extra.ins.sync_info = mybir.SyncInfo(on_wait=[w], on_update=[])
                else:
                    esi.on_wait = [w]
        nc.sync.drain()
        nc.all_engine_barrier()
        assert self.sems is not None
        popped = nc._tile_sem_poison_stack.pop()
        assert popped is self._sem_poison
        nc.clear_and_free_semaphores(list(self.sems.allocated().values()))
        nc.all_engine_barrier()


def _split_excess_waits(nc, max_waits=1):
    """This container's walrus build rejects instructions carrying more than
    one embedded sync wait. Move excess waits onto dedicated NOPs inserted
    just before the instruction on the same engine."""
    ctr = [0]
    for func in nc.m.functions:
        for block in func.blocks:
            out = []
            for inst in block.instructions:
                si = inst.sync_info
                waits = list(si.on_wait) if si and si.on_wait else []
                limit = 0 if isinstance(inst, mybir.InstDrain) else max_waits
                if len(waits) > limit:
                    keep = waits[:limit]
                    extra = waits[limit:]
                    for w in extra:
                        ctr[0] += 1
                        nop = mybir.InstNoOp(
                            name=f"waitnop-{ctr[0]}",
                            sync_info=mybir.SyncInfo(on_wait=[w], on_update=[]),
                            bass_nofuse=True,
                            engine=inst.engine,
                        )
                        out.append(nop)
                    si.on_wait = keep
                out.append(inst)
            block.instructions = out


def _rope_tables():
    keep = HD // 4  # 16 active frequencies; dims 16:32 of each half are identity
    active = (1.0 / ROPE_BASE) ** np.linspace(0.0, 1.0, keep, dtype=np.float32)
    th = np.arange(T, dtype=np.float32)[:, None] * active[None, :]
    return np.cos(th).astype(np.float32), np.sin(th).astype(np.float32)


def _classify_blocks(mask):
    """mask [T,T] additive, indexed (t, s). Block = (s-tile i of 128) x
    (t-block jb of 512). Returns cls[i][jb] in {skip, pass, tri, gen}."""
    cls = []
    for i in range(NT):
        row = []
        for jb in range(NJ):
            blk = mask[512 * jb:512 * (jb + 1), 128 * i:128 * (i + 1)]
            big_neg = blk <= -1e8
            zero = blk == 0.0
            if big_neg.all():
                row.append("skip")
            elif zero.all():
                row.append("pass")
            elif (big_neg | zero).all():
                tt = np.arange(512 * jb, 512 * (jb + 1))[:, None]
                ss = np.arange(128 * i, 128 * (i + 1))[None, :]
                row.append("tri" if np.array_equal(zero, tt >= ss) else "gen")
            else:
                row.append("gen")
        cls.append(row)
    return cls


def _build_program(cls, has_gen):
    nc = bass.Bass()
    xT = nc.declare_dram_parameter("xT", [DIM, T], BF16, isOutput=False)
    wq = nc.declare_dram_parameter("wqkvT", [DIM, 3 * GD], BF16, isOutput=False)
    wo = nc.declare_dram_parameter("woT", [GD, DIM], BF16, isOutput=False)
    ve = nc.declare_dram_parameter("ve_s", [T, GD], BF16, isOutput=False)
    lam = nc.declare_dram_parameter("lam0", [128, 1], F32, isOutput=False)
    # rope tables duplicated x2 per token so the (tile, q/k) dims merge into
    # one uniform-stride AP dim during the batched rope
    rC = nc.declare_dram_parameter("ropeC", [T, 32], BF16, isOutput=False)
    rS = nc.declare_dram_parameter("ropeS", [T, 32], BF16, isOutput=False)
    tri = nc.declare_dram_parameter("tri128", [128, 128], BF16, isOutput=False)
    md = None
    if has_gen:
        md = nc.declare_dram_parameter("maskdiv", [T, T], F32, isOutput=False)
    outT = nc.declare_dram_parameter("outT", [DIM, T], BF16, isOutput=True)

    # per-jb first/last valid s-tile for AV accumulation start/stop
    first_i = [None] * NJ
    last_i = [None] * NJ
    for jb in range(NJ):
        valid = [i for i in range(NT) if cls[i][jb] != "skip"]
        if valid:
            first_i[jb] = valid[0]
            last_i[jb] = valid[-1]

    def trim(i, jb):
        """Column offset (within the 512-block) of the valid range start."""
        if TRIM_DIAG and cls[i][jb] == "tri":
            return 128 * (i % 4)
        return 0

    with _TileContextFixed(nc) as tc, ExitStack() as ctx:
        S = ctx.enter_context(tc.tile_pool(name="singles", bufs=1))
        W = ctx.enter_context(tc.tile_pool(name="work", bufs=1))
        PS = ctx.enter_context(tc.tile_pool(name="ps", bufs=1, space="PSUM"))

        # ---- ACT table preload: one dummy exp so natural_log_exp loads at t=0
        dumm = S.tile([128, 1], F32, tag="dumm")
        nc.vector.memset(dumm, 0.0)
        dumo = S.tile([128, 1], F32, tag="dumo")
        nc.scalar.activation(dumo, dumm, mybir.ActivationFunctionType.Exp)
        dumo2 = S.tile([128, 1], F32, tag="dumo2")
        nc.scalar.activation(dumo2, dumm, mybir.ActivationFunctionType.Ln,
                             bias=dumm[:, 0:1], scale=1.0)

        # ---- weight / constant preloads
        wq_sb = S.tile([128, 8, 3 * GD], BF16, tag="wq_sb")
        for ds in range(8):
            nc.sync.dma_start(out=wq_sb[:, ds, :],
                              in_=wq[128 * ds:128 * (ds + 1), :])
        lam_sb = S.tile([128, 1], F32, tag="lam_sb")
        nc.sync.dma_start(out=lam_sb, in_=lam[:, :])
        rc_sb = S.tile([128, NT, 2, 16], BF16, tag="rc_sb")
        nc.sync.dma_start(out=rc_sb.rearrange("p a s k -> p a (s k)"),
                          in_=rC.rearrange("(a p) k -> p a k", p=128))
        rs_sb = S.tile([128, NT, 2, 16], BF16, tag="rs_sb")
        nc.sync.dma_start(out=rs_sb.rearrange("p a s k -> p a (s k)"),
                          in_=rS.rearrange("(a p) k -> p a k", p=128))
        tri_sb = S.tile([128, 128], BF16, tag="tri_sb")
        nc.sync.dma_start(out=tri_sb, in_=tri[:, :])
        wo_sb = S.tile([128, 2, DIM], BF16, tag="wo_sb")
        for dc in range(2):
            nc.sync.dma_start(out=wo_sb[:, dc, :],
                              in_=wo[128 * dc:128 * (dc + 1), :])

        eps_sb = S.tile([128, 1], F32, tag="eps_sb")
        nc.vector.memset(eps_sb, EPS)
        ident = S.tile([128, 128], BF16, tag="ident")
        from concourse.masks import make_identity
        make_identity(nc, ident)

        # v_aug[p, tt, h, 0:128]: even h -> [v | ones], odd h -> [ones | v]
        v_aug = S.tile([128, NT, HPG, 128], BF16, tag="v_aug")
        v5 = v_aug.rearrange("p t (a b) c -> p t a b c", b=2)
        nc.vector.memset(v5[:, :, :, 0, 64:128], 1.0)
        nc.vector.memset(v5[:, :, :, 1, 0:64], 1.0)

        qT = [S.tile([128, T], BF16, tag=f"qT{p}", name=f"qT{p}") for p in range(2)]
        kT = [S.tile([128, T], BF16, tag=f"kT{p}", name=f"kT{p}") for p in range(2)]
        yT = [S.tile([128, T], BF16, tag=f"yT{p}", name=f"yT{p}") for p in range(2)]

        pend_tp = []  # deferred transpose emissions: (tpb_view, qkb, u)

        def emit_transposes():
            if not pend_tp:
                return
            tpv, qkb, u = pend_tp.pop()
            qkb2 = qkb.rearrange("p (j c) -> p j c", j=2)
            for ec8 in range(8):
                j, ec = ec8 // 4, ec8 % 4
                nc.tensor.transpose(
                    tpv[:, 128 * ec8:128 * (ec8 + 1)],
                    qkb2[:, j, 128 * ec:128 * (ec + 1)], ident)
            tt0 = 2 * u
            for ec8 in range(8):
                j, ec = ec8 // 4, ec8 % 4
                dst = (qT if ec < 2 else kT)[ec % 2]
                dsl = dst[:, 128 * (tt0 + j):128 * (tt0 + j + 1)]
                src = tpv[:, 128 * ec8:128 * (ec8 + 1)]
                if u < 4:
                    nc.scalar.copy(dsl, src)
                else:
                    nc.vector.tensor_copy(out=dsl, in_=src)

        # ================= stage A: QKV + norm + rope (per 2-tile pair) =====
        def stage_a_pair(u):
            tts = (2 * u, 2 * u + 1)
            xt = W.tile([128, 2, 8, 128], BF16, tag="xt", bufs=2, name=f"xt{u}")
            for j, tt in enumerate(tts):
                nc.sync.dma_start(
                    out=xt[:, j],
                    in_=xT[:, 128 * tt:128 * (tt + 1)].rearrange(
                        "(s p) t -> p s t", p=128))
            ve_sb = W.tile([128, 2, GD], BF16, tag="ve", bufs=2, name=f"ve{u}")
            nc.sync.dma_start(
                out=ve_sb,
                in_=ve[256 * u:256 * (u + 1), :].rearrange(
                    "(j p) g -> p j g", p=128))

            qkvb = W.tile([128, 2, 3 * GD], BF16, tag="qkvb", bufs=2,
                          name=f"qkvb{u}")
            for j, tt in enumerate(tts):
                qp = PS.tile([128, 1024], F32, tag="sc", bufs=2, name=f"qp{tt}")
                for ds in range(8):
                    lhsT = xt[:, j, ds, :]
                    nc.tensor.matmul(
                        qp[:, 0:512], lhsT, wq_sb[:, ds, 0:512],
                        start=(ds == 0), stop=(ds == 7))
                    nc.tensor.matmul(
                        qp[:, 512:768], lhsT, wq_sb[:, ds, 512:768],
                        start=(ds == 0), stop=(ds == 7))
                if u < 4:
                    nc.scalar.copy(qkvb[:, j, :], qp[:, 0:768])
                else:
                    nc.vector.tensor_copy(out=qkvb[:, j, :], in_=qp[:, 0:768])

            # transposes of the PREVIOUS pair (deps long satisfied -> PE dense)
            emit_transposes()

            # rms stats: ms = sum(x^2) per head; rstd = exp(-0.5*ln(ms/64+eps))
            sq = W.tile([128, 2, 3 * GD], BF16, tag="sq", bufs=2, name=f"sq{u}")
            nc.vector.tensor_tensor(sq, qkvb, qkvb, mybir.AluOpType.mult)
            ms = W.tile([128, 2, 12], F32, tag="ms", bufs=2, name=f"ms{u}")
            nc.vector.tensor_reduce(
                ms, sq.rearrange("p j (g d) -> p j g d", d=HD),
                axis=mybir.AxisListType.X, op=mybir.AluOpType.add)
            lms = W.tile([128, 2, 12], F32, tag="lms", bufs=2, name=f"lms{u}")
            nc.scalar.activation(
                lms, ms, mybir.ActivationFunctionType.Ln,
                bias=eps_sb[:, 0:1], scale=1.0 / HD)
            rstd_bf = W.tile([128, 2, 12], BF16, tag="rstd", bufs=2,
                             name=f"rstd{u}")
            nc.scalar.activation(
                rstd_bf, lms, mybir.ActivationFunctionType.Exp, scale=-0.5)

            qkb = W.tile([128, 2 * 2 * GD], BF16, tag="qkb", bufs=2,
                         name=f"qkb{u}")
            nc.vector.tensor_tensor(
                qkb.rearrange("p (j g d) -> p j g d", j=2, d=HD),
                qkvb[:, :, 0:2 * GD].rearrange("p j (g d) -> p j g d", d=HD),
                rstd_bf[:, :, 0:8, None].to_broadcast([128, 2, 8, HD]),
                mybir.AluOpType.mult)
            vt = W.tile([128, 2, GD], BF16, tag="vt", bufs=2, name=f"vt{u}")
            nc.vector.tensor_tensor(
                vt.rearrange("p j (g d) -> p j g d", d=HD),
                qkvb[:, :, 2 * GD:3 * GD].rearrange("p j (g d) -> p j g d", d=HD),
                rstd_bf[:, :, 8:12, None].to_broadcast([128, 2, 4, HD]),
                mybir.AluOpType.mult)

            # blend: v_aug slot = lam0 * vt + ve_pre (ve pre-scaled by lam1)
            vt4 = vt.rearrange("p j (a b d) -> p j a b d", a=2, d=HD)
            vl4 = ve_sb.rearrange("p j (a b d) -> p j a b d", a=2, d=HD)
            nc.vector.scalar_tensor_tensor(
                out=v5[:, 2 * u:2 * u + 2, :, 0, 0:64], in0=vt4[:, :, :, 0, :],
                scalar=lam_sb[:, 0:1], in1=vl4[:, :, :, 0, :],
                op0=mybir.AluOpType.mult, op1=mybir.AluOpType.add)
            nc.vector.scalar_tensor_tensor(
                out=v5[:, 2 * u:2 * u + 2, :, 1, 64:128],
                in0=vt4[:, :, :, 1, :],
                scalar=lam_sb[:, 0:1], in1=vl4[:, :, :, 1, :],
                op0=mybir.AluOpType.mult, op1=mybir.AluOpType.add)

            # rope (active dims 0:16 and 32:48 of each head; q and k together;
            # a = (tile j, q/k s) merged into one uniform-stride dim of 4)
            v6 = qkb.rearrange("p (a g d) -> p a g d", a=4, d=HD)
            x0 = v6[:, :, :, 0:16]
            x32 = v6[:, :, :, 32:48]
            rcf = rc_sb.rearrange("p a s k -> p (a s) k")
            rsf = rs_sb.rearrange("p a s k -> p (a s) k")
            cb = rcf[:, 4 * u:4 * u + 4, None, :].to_broadcast(
                [128, 4, HPG, 16])
            sb = rsf[:, 4 * u:4 * u + 4, None, :].to_broadcast(
                [128, 4, HPG, 16])
            ra = W.tile([128, 4, HPG, 16], BF16, tag="ra", bufs=2,
                        name=f"ra{u}")
            rb = W.tile([128, 4, HPG, 16], BF16, tag="rb", bufs=2,
                        name=f"rb{u}")
            nc.vector.tensor_tensor(ra, x0, sb, mybir.AluOpType.mult)
            nc.vector.tensor_tensor(rb, x32, sb, mybir.AluOpType.mult)
            nc.vector.tensor_tensor(x0, x0, cb, mybir.AluOpType.mult)
            nc.vector.tensor_tensor(x32, x32, cb, mybir.AluOpType.mult)
            nc.vector.tensor_tensor(x0, x0, rb, mybir.AluOpType.add)
            nc.vector.tensor_tensor(x32, x32, ra, mybir.AluOpType.subtract)

            # transposes run next pair (or via flush); out via PSUM bf16 view
            tpb = PS.tile([128, 1024], F32, tag="av", bufs=2, name=f"tpb{u}")
            pend_tp.append((tpb.bitcast(BF16)[:, 0:1024], qkb, u))

        # ================= stage B: attention for one (pair, jj) ============
        def stage_b(pair, jj):
            h0, h1 = 2 * pair, 2 * pair + 1
            av0 = PS.tile([128, 1024], F32, tag="av", bufs=2,
                          name=f"av{pair}_{jj}_0")
            av1 = PS.tile([128, 1024], F32, tag="av", bufs=2,
                          name=f"av{pair}_{jj}_1")
            avs = (av0, av1)
            for i in range(NT):
                jbs = [jb for jb in (2 * jj, 2 * jj + 1)
                       if cls[i][jb] != "skip"]
                if not jbs:
                    continue
                sc0 = PS.tile([128, 1024], F32, tag="sc", bufs=2,
                              name=f"sc{pair}_{jj}_{i}_0")
                sc1 = PS.tile([128, 1024], F32, tag="sc", bufs=2,
                              name=f"sc{pair}_{jj}_{i}_1")
                scs = (sc0, sc1)
                # interleave T0/T8 row-tiles so they run concurrently
                for jb in jbs:
                    off = 512 * (jb - 2 * jj) + trim(i, jb)
                    w = 512 - trim(i, jb)
                    for hh in range(2):
                        rlo = 64 * hh
                        nc.tensor.matmul(
                            scs[hh][:, off:off + w],
                            kT[pair][rlo:rlo + 64, 128 * i:128 * (i + 1)],
                            qT[pair][rlo:rlo + 64,
                                     512 * jb + trim(i, jb):512 * (jb + 1)],
                            start=True, stop=True)
                if has_gen:
                    for jb in jbs:
                        if cls[i][jb] != "gen":
                            continue
                        off = 512 * (jb - 2 * jj)
                        mdt = W.tile([128, 512], F32, tag="mdt", bufs=2,
                                     name=f"mdt{pair}_{jj}_{i}_{jb}")
                        nc.sync.dma_start(
                            out=mdt,
                            in_=md[512 * jb:512 * (jb + 1),
                                   128 * i:128 * (i + 1)].rearrange("t s -> s t"))
                        for hh in range(2):
                            nc.vector.tensor_tensor(
                                scs[hh][:, off:off + 512],
                                scs[hh][:, off:off + 512],
                                mdt, mybir.AluOpType.add)
                lo = 512 * (min(jbs) - 2 * jj) + trim(i, min(jbs))
                hi = 512 * (max(jbs) - 2 * jj) + 512
                exs = []
                for hh in range(2):
                    ex = W.tile([128, 1024], BF16, tag="ex", bufs=4,
                                name=f"ex{pair}_{jj}_{i}_{hh}")
                    nc.scalar.activation(
                        ex[:, lo:hi], scs[hh][:, lo:hi],
                        mybir.ActivationFunctionType.Exp, scale=SCALE)
                    exs.append(ex)
                for jb in jbs:
                    if cls[i][jb] != "tri":
                        continue
                    off = 512 * (jb - 2 * jj) + trim(i, jb)
                    tw = 128 if TRIM_DIAG else 512 - trim(i, jb)
                    msk = (tri_sb[:, 0:tw] if TRIM_DIAG
                           else tri_sb[:, 0:128])
                    for hh in range(2):
                        eng = nc.gpsimd if GPSIMD_TRI else nc.vector
                        eng.tensor_tensor(
                            exs[hh][:, off:off + tw],
                            exs[hh][:, off:off + tw],
                            msk, mybir.AluOpType.mult)
                for hh, h in ((0, h0), (1, h1)):
                    for jb in jbs:
                        off = 512 * (jb - 2 * jj) + trim(i, jb)
                        w = 512 - trim(i, jb)
                        nc.tensor.matmul(
                            avs[hh][:, off:off + w],
                            v_aug[:, i, h, :],
                            exs[hh][:, off:off + w],
                            start=(first_i[jb] == i),
                            stop=(last_i[jb] == i))
            # empty column-blocks (pathological masks): den=1, y=0
            for jb in (2 * jj, 2 * jj + 1):
                if first_i[jb] is None:
                    off = 512 * (jb - 2 * jj)
                    for a in avs:
                        nc.vector.memset(a[:, off:off + 512], 1.0)
            # normalize: av0 = [y0 | den0-rep], av1 = [den1-rep | y1]
            hsl = slice(1024 * jj, 1024 * (jj + 1))
            rec = W.tile([128, 2, 1024], F32, tag="rec", bufs=2,
                         name=f"rec{pair}_{jj}")
            nc.vector.reciprocal(rec[64:128, 0, :], av0[64:128, :])
            nc.vector.reciprocal(rec[0:64, 1, :], av1[0:64, :])
            nc.sync.dma_start(out=rec[0:64, 0, :], in_=rec[64:128, 0, :])
            nc.sync.dma_start(out=rec[64:128, 1, :], in_=rec[0:64, 1, :])
            nc.vector.tensor_tensor(
                yT[pair][0:64, hsl], av0[0:64, :], rec[0:64, 0, :],
                mybir.AluOpType.mult)
            nc.vector.tensor_tensor(
                yT[pair][64:128, hsl], av1[64:128, :], rec[64:128, 1, :],
                mybir.AluOpType.mult)

        # ================= stage C: output projection for one 512-col tb ====
        def stage_c(tb):
            for ec in range(8):
                po = PS.tile([128, 1024], F32, tag="sc", bufs=2,
                             name=f"po{tb}_{ec}")
                for dc in range(2):
                    nc.tensor.matmul(
                        po[:, 0:512],
                        wo_sb[:, dc, 128 * ec:128 * (ec + 1)],
                        yT[dc][:, 512 * tb:512 * (tb + 1)],
                        start=(dc == 0), stop=(dc == 1))
                os = W.tile([128, 512], BF16, tag="os", bufs=4,
                            name=f"os{tb}_{ec}")
                if ec % 2 == 0:
                    nc.scalar.copy(os, po[:, 0:512])
                else:
                    nc.vector.tensor_copy(out=os, in_=po[:, 0:512])
                nc.sync.dma_start(
                    out=outT[128 * ec:128 * (ec + 1),
                             512 * tb:512 * (tb + 1)],
                    in_=os)

        # ================= emission schedule ================================
        for u in range(4):
            stage_a_pair(u)
        emit_transposes()
        stage_b(0, 0)
        stage_a_pair(4)
        stage_a_pair(5)
        stage_b(1, 0)
        stage_a_pair(6)
        stage_a_pair(7)
        emit_transposes()
        stage_c(0)
        stage_c(1)
        stage_b(0, 1)
        stage_b(1, 1)
        stage_c(2)
        stage_c(3)
    _split_excess_waits(nc)
    return nc


def kernel(x, ve, sa_lambdas, attn_mask, qkvo_w):
    global LAST_EXEC_NS
    x = np.ascontiguousarray(np.asarray(x, np.float32))
    ve = np.ascontiguousarray(np.asarray(ve, np.float32))
    sa_lambdas = np.asarray(sa_lambdas, np.float32)
    attn_mask = np.asarray(attn_mask, np.float32)
    qkvo_w = np.asarray(qkvo_w, np.float32)

    ropeC, ropeS = _rope_tables()
    mask = attn_mask[0, 0]
    cls = _classify_blocks(mask)
    has_gen = any(c == "gen" for row in cls for c in row)

    nc = _build_program(cls, has_gen)

    part = np.arange(128)
    tri128 = (part[None, :] >= part[:, None]).astype(ml_dtypes.bfloat16)
    lam0 = np.full((128, 1), sa_lambdas[0], np.float32)
    rCb = np.ascontiguousarray(
        np.tile(ropeC[:, None, :], (1, 2, 1)).reshape(T, 32)
    ).astype(ml_dtypes.bfloat16)
    rSb = np.ascontiguousarray(
        np.tile(ropeS[:, None, :], (1, 2, 1)).reshape(T, 32)
    ).astype(ml_dtypes.bfloat16)
    maskdiv = (mask / SCALE).astype(np.float32) if has_gen else None

    in_maps = []
    for c in range(8):
        b, g = c // G, c % G
        sl = slice(GD * g, GD * (g + 1))
        wqkvT = np.ascontiguousarray(
            np.concatenate([qkvo_w[k][sl, :] for k in range(3)], 0).T)
        m = {
            "xT": np.ascontiguousarray(x[b].T).astype(ml_dtypes.bfloat16),
            "wqkvT": wqkvT.astype(ml_dtypes.bfloat16),
            "woT": np.ascontiguousarray(
                qkvo_w[3][:, sl].T).astype(ml_dtypes.bfloat16),
            "ve_s": np.ascontiguousarray(
                ve[b][:, sl] * sa_lambdas[1]).astype(ml_dtypes.bfloat16),
            "lam0": lam0,
            "ropeC": rCb,
            "ropeS": rSb,
            "tri128": tri128,
        }
        if has_gen:
            m["maskdiv"] = maskdiv
        in_maps.append(m)

    res = run_bass_kernel_spmd(nc, in_maps, core_ids=list(range(8)),
                               trace=TRACE)
    if TRACE:
        LAST_EXEC_NS = res.exec_time_ns

    out = np.zeros((B, T, DIM), np.float32)
    for c in range(8):
        out[c // G] += res.results[c]["outT"].T.astype(np.float32)
    return out


# revision 14
# speedup vs baseline: 1.2417x; 1.1261x over previous
"""Trainium2 Bass kernel for nn_CausalSelfAttention_28467043237962.

Sharding: 8 cores = 2 batches x 4 head-groups (4 heads / 256 dims each).
Per core: fused QKV projection (bf16 matmuls, fp32 PSUM), per-head RMS-norm
(rstd = exp(-0.5*ln(ms)) so one ACT table set serves norm AND softmax),
RoPE on the 16 active frequencies, causal attention in transposed-scores
layout (scores_T[s,t]; denominators from [v|ones] columns in the AV matmul,
normalized by a direct per-partition reciprocal + one SBUF->SBUF DMA to
shift partitions). Diagonal 128x512 blocks are column-trimmed to the valid
range with a single upper-tri 128x128 mask. Stage emission is interleaved
(QKV tiles 8-15 run while jj=0 softmax exps drain) to keep the PE dense and
HAM-warm. Output projection emits bf16 partials; host sums the 4 groups.

Self-contained: builds one SPMD Bass program and runs it on cores 0-7 via
concourse.bass_utils.run_bass_kernel_spmd.
"""
import sys

sys.path.insert(0, "/opt/trn_rl_repo")

from contextlib import ExitStack

import numpy as np
import ml_dtypes

import concourse.bass as bass
import concourse.tile as tile
import concourse.mybir as mybir
from concourse.vector_clock import ScopedClock
from concourse.bass_utils import run_bass_kernel_spmd

F32 = mybir.dt.float32
BF16 = mybir.dt.bfloat16

B, T, DIM = 2, 2048, 1024
H, HD = 16, 64
SCALE = 0.12
ROPE_BASE = 1024.0
EPS = 1e-6
G = 4          # head-groups = cores per batch
HPG = 4        # heads per group
GD = HPG * HD  # 256 dims per group
NT = T // 128  # 16 s/t tiles of 128
NJ = T // 512  # 4 t-blocks of 512

TRACE = False          # set by test.py for profiling runs
LAST_EXEC_NS = None    # filled when TRACE

TRIM_DIAG = True       # column-trim diagonal blocks to the causal range
GPSIMD_TRI = True      # run the tri-square masks on the GpSimd engine


class _TileContextFixed(tile.TileContext):
    """Workaround for this container's walrus build: the kernel-tail InstDrain
    may carry only one sync wait. Spread the tail waits over single-wait NOPs
    on the sync engine before a wait-free drain."""

    def _drain_and_barrier(self, tick_clock, wait_clock):
        nc = self.nc
        collector = nc.sync.nop(nofuse=True, hint="tail_wait_collector")
        wait_clock.add_sem_waits(
            collector.ins, ScopedClock({None: tick_clock.global_clock})
        )
        si = collector.ins.sync_info
        waits = list(si.on_wait or [])
        if len(waits) > 1:
            si.on_wait = waits[:1]
            for w in waits[1:]:
                extra = nc.sync.nop(nofuse=True, hint="tail_wait")
                esi = extra.ins.sync_info
                if esi is None:
                    extra.ins.sync_info = mybir.SyncInfo(on_wait=[w], on_update=[])
                else:
                    esi.on_wait = [w]
        nc.sync.drain()
        nc.all_engine_barrier()
        assert self.sems is not None
        popped = nc._tile_sem_poison_stack.pop()
        assert popped is self._sem_poison
        nc.clear_and_free_semaphores(list(self.sems.allocated().values()))
        nc.all_engine_barrier()


def _split_excess_waits(nc, max_waits=1):
    """This container's walrus build rejects instructions carrying more than
    one embedded sync wait. Move excess waits onto dedicated NOPs inserted
    just before the instruction on the same engine."""
    ctr = [0]
    for func in nc.m.functions:
        for block in func.blocks:
            out = []
            for inst in block.instructions:
                si = inst.sync_info
                waits = list(si.on_wait) if si and si.on_wait else []
                limit = 0 if isinstance(inst, mybir.InstDrain) else max_waits
                if len(waits) > limit:
                    keep = waits[:limit]
                    extra = waits[limit:]
                    for w in extra:
                        ctr[0] += 1
                        nop = mybir.InstNoOp(
                            name=f"waitnop-{ctr[0]}",
                            sync_info=mybir.SyncInfo(on_wait=[w], on_update=[]),
                            bass_nofuse=True,
                            engine=inst.engine,
                        )
                        out.append(nop)
                    si.on_wait = keep
                out.append(inst)
            block.instructions = out


def _rope_tables():
    keep = HD // 4  # 16 active frequencies; dims 16:32 of each half are identity
    active = (1.0 / ROPE_BASE) ** np.linspace(0.0, 1.0, keep, dtype=np.float32)
    th = np.arange(T, dtype=np.float32)[:, None] * active[None, :]
    return np.cos(th).astype(np.float32), np.sin(th).astype(np.float32)


def _classify_blocks(mask):
    """mask [T,T] additive, indexed (t, s). Block = (s-tile i of 128) x
    (t-block jb of 512). Returns cls[i][jb] in {skip, pass, tri, gen}."""
    cls = []
    for i in range(NT):
        row = []
        for jb in range(NJ):
            blk = mask[512 * jb:512 * (jb + 1), 128 * i:128 * (i + 1)]
            big_neg = blk <= -1e8
            zero = blk == 0.0
            if big_neg.all():
                row.append("skip")
            elif zero.all():
                row.append("pass")
            elif (big_neg | zero).all():
                tt = np.arange(512 * jb, 512 * (jb + 1))[:, None]
                ss = np.arange(128 * i, 128 * (i + 1))[None, :]
                row.append("tri" if np.array_equal(zero, tt >= ss) else "gen")
            else:
                row.append("gen")
        cls.append(row)
    return cls


def _build_program(cls, has_gen):
    nc = bass.Bass()
    xT = nc.declare_dram_parameter("xT", [DIM, T], BF16, isOutput=False)
    wq = nc.declare_dram_parameter("wqkvT", [DIM, 3 * GD], BF16, isOutput=False)
    wo = nc.declare_dram_parameter("woT", [GD, DIM], BF16, isOutput=False)
    ve = nc.declare_dram_parameter("ve_s", [T, GD], BF16, isOutput=False)
    lam = nc.declare_dram_parameter("lam0", [128, 1], F32, isOutput=False)
    # rope tables duplicated x2 per token so the (tile, q/k) dims merge into
    # one uniform-stride AP dim during the batched rope
    rC = nc.declare_dram_parameter("ropeC", [T, 32], BF16, isOutput=False)
    rS = nc.declare_dram_parameter("ropeS", [T, 32], BF16, isOutput=False)
    tri = nc.declare_dram_parameter("tri128", [128, 128], BF16, isOutput=False)
    md = None
    if has_gen:
        md = nc.declare_dram_parameter("maskdiv", [T, T], F32, isOutput=False)
    outT = nc.declare_dram_parameter("outT", [DIM, T], BF16, isOutput=True)

    # per-jb first/last valid s-tile for AV accumulation start/stop
    first_i = [None] * NJ
    last_i = [None] * NJ
    for jb in range(NJ):
        valid = [i for i in range(NT) if cls[i][jb] != "skip"]
        if valid:
            first_i[jb] = valid[0]
            last_i[jb] = valid[-1]

    def trim(i, jb):
        """Column offset (within the 512-block) of the valid range start."""
        if TRIM_DIAG and cls[i][jb] == "tri":
            return 128 * (i % 4)
        return 0

    with _TileContextFixed(nc) as tc, ExitStack() as ctx:
        S = ctx.enter_context(tc.tile_pool(name="singles", bufs=1))
        W = ctx.enter_context(tc.tile_pool(name="work", bufs=1))
        PS = ctx.enter_context(tc.tile_pool(name="ps", bufs=1, space="PSUM"))

        # ---- ACT table preload: one dummy exp so natural_log_exp loads at t=0
        dumm = S.tile([128, 1], F32, tag="dumm")
        nc.vector.memset(dumm, 0.0)
        dumo = S.tile([128, 1], F32, tag="dumo")
        nc.scalar.activation(dumo, dumm, mybir.ActivationFunctionType.Exp)
        dumo2 = S.tile([128, 1], F32, tag="dumo2")
        nc.scalar.activation(dumo2, dumm, mybir.ActivationFunctionType.Ln,
                             bias=dumm[:, 0:1], scale=1.0)

        # ---- weight / constant preloads
        wq_sb = S.tile([128, 8, 3 * GD], BF16, tag="wq_sb")
        for ds in range(8):
            nc.sync.dma_start(out=wq_sb[:, ds, :],
                              in_=wq[128 * ds:128 * (ds + 1), :])
        # bulk constants ride the (idle) gpsimd DMA queue so the sync queue
        # serves the stage-A xt/ve stream without head-of-line blocking
        lam_sb = S.tile([128, 1], F32, tag="lam_sb")
        nc.gpsimd.dma_start(out=lam_sb, in_=lam[:, :])
        rc_sb = S.tile([128, NT, 2, 16], BF16, tag="rc_sb")
        nc.gpsimd.dma_start(out=rc_sb.rearrange("p a s k -> p a (s k)"),
                            in_=rC.rearrange("(a p) k -> p a k", p=128))
        rs_sb = S.tile([128, NT, 2, 16], BF16, tag="rs_sb")
        nc.gpsimd.dma_start(out=rs_sb.rearrange("p a s k -> p a (s k)"),
                            in_=rS.rearrange("(a p) k -> p a k", p=128))
        tri_sb = S.tile([128, 128], BF16, tag="tri_sb")
        nc.gpsimd.dma_start(out=tri_sb, in_=tri[:, :])
        wo_sb = S.tile([128, 2, DIM], BF16, tag="wo_sb")
        for dc in range(2):
            nc.gpsimd.dma_start(out=wo_sb[:, dc, :],
                                in_=wo[128 * dc:128 * (dc + 1), :])

        eps_sb = S.tile([128, 1], F32, tag="eps_sb")
        nc.vector.memset(eps_sb, EPS)
        ident = S.tile([128, 128], BF16, tag="ident")
        from concourse.masks import make_identity
        make_identity(nc, ident)

        # v_aug[p, tt, h, 0:128]: even h -> [v | ones], odd h -> [ones | v]
        v_aug = S.tile([128, NT, HPG, 128], BF16, tag="v_aug")
        v5 = v_aug.rearrange("p t (a b) c -> p t a b c", b=2)
        nc.vector.memset(v5[:, :, :, 0, 64:128], 1.0)
        nc.vector.memset(v5[:, :, :, 1, 0:64], 1.0)

        qT = [S.tile([128, T], BF16, tag=f"qT{p}", name=f"qT{p}") for p in range(2)]
        kT = [S.tile([128, T], BF16, tag=f"kT{p}", name=f"kT{p}") for p in range(2)]
        yT = [S.tile([128, T], BF16, tag=f"yT{p}", name=f"yT{p}") for p in range(2)]

        pend_tp = []  # deferred transpose emissions: (tpb_view, qkb, u)

        def emit_transposes():
            if not pend_tp:
                return
            tpv, qkb, u = pend_tp.pop()
            qkb2 = qkb.rearrange("p (j c) -> p j c", j=2)
            for ec8 in range(8):
                j, ec = ec8 // 4, ec8 % 4
                nc.tensor.transpose(
                    tpv[:, 128 * ec8:128 * (ec8 + 1)],
                    qkb2[:, j, 128 * ec:128 * (ec + 1)], ident)
            tt0 = 2 * u
            for ec8 in range(8):
                j, ec = ec8 // 4, ec8 % 4
                dst = (qT if ec < 2 else kT)[ec % 2]
                dsl = dst[:, 128 * (tt0 + j):128 * (tt0 + j + 1)]
                src = tpv[:, 128 * ec8:128 * (ec8 + 1)]
                if u < 4:
                    nc.scalar.copy(dsl, src)
                else:
                    nc.vector.tensor_copy(out=dsl, in_=src)

        # ================= stage A: QKV + norm + rope (per 2-tile pair) =====
        def stage_a_pair(u):
            tts = (2 * u, 2 * u + 1)
            xt = W.tile([128, 2, 8, 128], BF16, tag="xt", bufs=2, name=f"xt{u}")
            for j, tt in enumerate(tts):
                nc.sync.dma_start(
                    out=xt[:, j],
                    in_=xT[:, 128 * tt:128 * (tt + 1)].rearrange(
                        "(s p) t -> p s t", p=128))
            ve_sb = W.tile([128, 2, GD], BF16, tag="ve", bufs=2, name=f"ve{u}")
            nc.sync.dma_start(
                out=ve_sb,
                in_=ve[256 * u:256 * (u + 1), :].rearrange(
                    "(j p) g -> p j g", p=128))

            qkvb = W.tile([128, 2, 3 * GD], BF16, tag="qkvb", bufs=2,
                          name=f"qkvb{u}")
            for j, tt in enumerate(tts):
                qp = PS.tile([128, 1024], F32, tag="sc", bufs=2, name=f"qp{tt}")
                for ds in range(8):
                    lhsT = xt[:, j, ds, :]
                    nc.tensor.matmul(
                        qp[:, 0:512], lhsT, wq_sb[:, ds, 0:512],
                        start=(ds == 0), stop=(ds == 7))
                    nc.tensor.matmul(
                        qp[:, 512:768], lhsT, wq_sb[:, ds, 512:768],
                        start=(ds == 0), stop=(ds == 7))
                if u < 4:
                    nc.scalar.copy(qkvb[:, j, :], qp[:, 0:768])
                else:
                    nc.vector.tensor_copy(out=qkvb[:, j, :], in_=qp[:, 0:768])

            # transposes of the PREVIOUS pair (deps long satisfied -> PE dense)
            emit_transposes()

            # rms stats: ms = sum(x^2) per head; rstd = exp(-0.5*ln(ms/64+eps))
            sq = W.tile([128, 2, 3 * GD], BF16, tag="sq", bufs=2, name=f"sq{u}")
            nc.vector.tensor_tensor(sq, qkvb, qkvb, mybir.AluOpType.mult)
            ms = W.tile([128, 2, 12], F32, tag="ms", bufs=2, name=f"ms{u}")
            nc.vector.tensor_reduce(
                ms, sq.rearrange("p j (g d) -> p j g d", d=HD),
                axis=mybir.AxisListType.X, op=mybir.AluOpType.add)
            lms = W.tile([128, 2, 12], F32, tag="lms", bufs=2, name=f"lms{u}")
            nc.scalar.activation(
                lms, ms, mybir.ActivationFunctionType.Ln,
                bias=eps_sb[:, 0:1], scale=1.0 / HD)
            rstd_bf = W.tile([128, 2, 12], BF16, tag="rstd", bufs=2,
                             name=f"rstd{u}")
            nc.scalar.activation(
                rstd_bf, lms, mybir.ActivationFunctionType.Exp, scale=-0.5)

            qkb = W.tile([128, 2 * 2 * GD], BF16, tag="qkb", bufs=2,
                         name=f"qkb{u}")
            nc.vector.tensor_tensor(
                qkb.rearrange("p (j g d) -> p j g d", j=2, d=HD),
                qkvb[:, :, 0:2 * GD].rearrange("p j (g d) -> p j g d", d=HD),
                rstd_bf[:, :, 0:8, None].to_broadcast([128, 2, 8, HD]),
                mybir.AluOpType.mult)
            vt = W.tile([128, 2, GD], BF16, tag="vt", bufs=2, name=f"vt{u}")
            nc.vector.tensor_tensor(
                vt.rearrange("p j (g d) -> p j g d", d=HD),
                qkvb[:, :, 2 * GD:3 * GD].rearrange("p j (g d) -> p j g d", d=HD),
                rstd_bf[:, :, 8:12, None].to_broadcast([128, 2, 4, HD]),
                mybir.AluOpType.mult)

            # blend: v_aug slot = lam0 * vt + ve_pre (ve pre-scaled by lam1)
            vt4 = vt.rearrange("p j (a b d) -> p j a b d", a=2, d=HD)
            vl4 = ve_sb.rearrange("p j (a b d) -> p j a b d", a=2, d=HD)
            nc.vector.scalar_tensor_tensor(
                out=v5[:, 2 * u:2 * u + 2, :, 0, 0:64], in0=vt4[:, :, :, 0, :],
                scalar=lam_sb[:, 0:1], in1=vl4[:, :, :, 0, :],
                op0=mybir.AluOpType.mult, op1=mybir.AluOpType.add)
            nc.vector.scalar_tensor_tensor(
                out=v5[:, 2 * u:2 * u + 2, :, 1, 64:128],
                in0=vt4[:, :, :, 1, :],
                scalar=lam_sb[:, 0:1], in1=vl4[:, :, :, 1, :],
                op0=mybir.AluOpType.mult, op1=mybir.AluOpType.add)

            # rope (active dims 0:16 and 32:48 of each head; q and k together;
            # a = (tile j, q/k s) merged into one uniform-stride dim of 4)
            v6 = qkb.rearrange("p (a g d) -> p a g d", a=4, d=HD)
            x0 = v6[:, :, :, 0:16]
            x32 = v6[:, :, :, 32:48]
            rcf = rc_sb.rearrange("p a s k -> p (a s) k")
            rsf = rs_sb.rearrange("p a s k -> p (a s) k")
            cb = rcf[:, 4 * u:4 * u + 4, None, :].to_broadcast(
                [128, 4, HPG, 16])
            sb = rsf[:, 4 * u:4 * u + 4, None, :].to_broadcast(
                [128, 4, HPG, 16])
            ra = W.tile([128, 4, HPG, 16], BF16, tag="ra", bufs=2,
                        name=f"ra{u}")
            rb = W.tile([128, 4, HPG, 16], BF16, tag="rb", bufs=2,
                        name=f"rb{u}")
            nc.vector.tensor_tensor(ra, x0, sb, mybir.AluOpType.mult)
            nc.vector.tensor_tensor(rb, x32, sb, mybir.AluOpType.mult)
            nc.vector.tensor_tensor(x0, x0, cb, mybir.AluOpType.mult)
            nc.vector.tensor_tensor(x32, x32, cb, mybir.AluOpType.mult)
            nc.vector.tensor_tensor(x0, x0, rb, mybir.AluOpType.add)
            nc.vector.tensor_tensor(x32, x32, ra, mybir.AluOpType.subtract)

            # transposes run next pair (or via flush); out via PSUM bf16 view
            tpb = PS.tile([128, 1024], F32, tag="av", bufs=2, name=f"tpb{u}")
            pend_tp.append((tpb.bitcast(BF16)[:, 0:1024], qkb, u))

        # ================= stage B: attention for one (pair, jj) ============
        def stage_b(pair, jj):
            h0, h1 = 2 * pair, 2 * pair + 1
            av0 = PS.tile([128, 1024], F32, tag="av", bufs=2,
                          name=f"av{pair}_{jj}_0")
            av1 = PS.tile([128, 1024], F32, tag="av", bufs=2,
                          name=f"av{pair}_{jj}_1")
            avs = (av0, av1)
            for i in range(NT):
                jbs = [jb for jb in (2 * jj, 2 * jj + 1)
                       if cls[i][jb] != "skip"]
                if not jbs:
                    continue
                sc0 = PS.tile([128, 1024], F32, tag="sc", bufs=2,
                              name=f"sc{pair}_{jj}_{i}_0")
                sc1 = PS.tile([128, 1024], F32, tag="sc", bufs=2,
                              name=f"sc{pair}_{jj}_{i}_1")
                scs = (sc0, sc1)
                # interleave T0/T8 row-tiles so they run concurrently
                for jb in jbs:
                    off = 512 * (jb - 2 * jj) + trim(i, jb)
                    w = 512 - trim(i, jb)
                    for hh in range(2):
                        rlo = 64 * hh
                        nc.tensor.matmul(
                            scs[hh][:, off:off + w],
                            kT[pair][rlo:rlo + 64, 128 * i:128 * (i + 1)],
                            qT[pair][rlo:rlo + 64,
                                     512 * jb + trim(i, jb):512 * (jb + 1)],
                            start=True, stop=True)
                if has_gen:
                    for jb in jbs:
                        if cls[i][jb] != "gen":
                            continue
                        off = 512 * (jb - 2 * jj)
                        mdt = W.tile([128, 512], F32, tag="mdt", bufs=2,
                                     name=f"mdt{pair}_{jj}_{i}_{jb}")
                        nc.sync.dma_start(
                            out=mdt,
                            in_=md[512 * jb:512 * (jb + 1),
                                   128 * i:128 * (i + 1)].rearrange("t s -> s t"))
                        for hh in range(2):
                            nc.vector.tensor_tensor(
                                scs[hh][:, off:off + 512],
                                scs[hh][:, off:off + 512],
                                mdt, mybir.AluOpType.add)
                lo = 512 * (min(jbs) - 2 * jj) + trim(i, min(jbs))
                hi = 512 * (max(jbs) - 2 * jj) + 512
                exs = []
                for hh in range(2):
                    ex = W.tile([128, 1024], BF16, tag="ex", bufs=4,
                                name=f"ex{pair}_{jj}_{i}_{hh}")
                    nc.scalar.activation(
                        ex[:, lo:hi], scs[hh][:, lo:hi],
                        mybir.ActivationFunctionType.Exp, scale=SCALE)
                    exs.append(ex)
                for jb in jbs:
                    if cls[i][jb] != "tri":
                        continue
                    off = 512 * (jb - 2 * jj) + trim(i, jb)
                    tw = 128 if TRIM_DIAG else 512 - trim(i, jb)
                    msk = (tri_sb[:, 0:tw] if TRIM_DIAG
                           else tri_sb[:, 0:128])
                    for hh in range(2):
                        eng = nc.gpsimd if GPSIMD_TRI else nc.vector
                        eng.tensor_tensor(
                            exs[hh][:, off:off + tw],
                            exs[hh][:, off:off + tw],
                            msk, mybir.AluOpType.mult)
                for hh, h in ((0, h0), (1, h1)):
                    for jb in jbs:
                        off = 512 * (jb - 2 * jj) + trim(i, jb)
                        w = 512 - trim(i, jb)
                        nc.tensor.matmul(
                            avs[hh][:, off:off + w],
                            v_aug[:, i, h, :],
                            exs[hh][:, off:off + w],
                            start=(first_i[jb] == i),
                            stop=(last_i[jb] == i))
            # empty column-blocks (pathological masks): den=1, y=0
            for jb in (2 * jj, 2 * jj + 1):
                if first_i[jb] is None:
                    off = 512 * (jb - 2 * jj)
                    for a in avs:
                        nc.vector.memset(a[:, off:off + 512], 1.0)
            # normalize: av0 = [y0 | den0-rep], av1 = [den1-rep | y1]
            hsl = slice(1024 * jj, 1024 * (jj + 1))
            # 1/den via exp(-ln(den)) on ACT: stays in the natural_log_exp
            # table set (DVE reciprocal is an iterative ~6.5us sequence)
            rec = W.tile([128, 3, 1024], F32, tag="rec", bufs=2,
                         name=f"rec{pair}_{jj}")
            nc.scalar.activation(rec[64:128, 2, :], av0[64:128, :],
                                 mybir.ActivationFunctionType.Ln)
            nc.scalar.activation(rec[64:128, 0, :], rec[64:128, 2, :],
                                 mybir.ActivationFunctionType.Exp, scale=-1.0)
            nc.scalar.activation(rec[0:64, 2, :], av1[0:64, :],
                                 mybir.ActivationFunctionType.Ln)
            nc.scalar.activation(rec[0:64, 1, :], rec[0:64, 2, :],
                                 mybir.ActivationFunctionType.Exp, scale=-1.0)
            nc.gpsimd.dma_start(out=rec[0:64, 0, :], in_=rec[64:128, 0, :])
            nc.gpsimd.dma_start(out=rec[64:128, 1, :], in_=rec[0:64, 1, :])
            nc.vector.tensor_tensor(
                yT[pair][0:64, hsl], av0[0:64, :], rec[0:64, 0, :],
                mybir.AluOpType.mult)
            nc.vector.tensor_tensor(
                yT[pair][64:128, hsl], av1[64:128, :], rec[64:128, 1, :],
                mybir.AluOpType.mult)

        # ================= stage C: output projection for one 512-col tb ====
        def stage_c(tb):
            for ec in range(8):
                po = PS.tile([128, 1024], F32, tag="sc", bufs=2,
                             name=f"po{tb}_{ec}")
                for dc in range(2):
                    nc.tensor.matmul(
                        po[:, 0:512],
                        wo_sb[:, dc, 128 * ec:128 * (ec + 1)],
                        yT[dc][:, 512 * tb:512 * (tb + 1)],
                        start=(dc == 0), stop=(dc == 1))
                os = W.tile([128, 512], BF16, tag="os", bufs=4,
                            name=f"os{tb}_{ec}")
                if ec % 2 == 0:
                    nc.scalar.copy(os, po[:, 0:512])
                else:
                    nc.vector.tensor_copy(out=os, in_=po[:, 0:512])
                nc.sync.dma_start(
                    out=outT[128 * ec:128 * (ec + 1),
                             512 * tb:512 * (tb + 1)],
                    in_=os)

        # ================= emission schedule ================================
        for u in range(4):
            stage_a_pair(u)
        emit_transposes()
        stage_b(0, 0)
        stage_a_pair(4)
        stage_a_pair(5)
        stage_b(1, 0)
        stage_a_pair(6)
        stage_a_pair(7)
        emit_transposes()
        stage_c(0)
        stage_c(1)
        stage_b(0, 1)
        stage_b(1, 1)
        stage_c(2)
        stage_c(3)
    _split_excess_waits(nc)
    return nc


def kernel(x, ve, sa_lambdas, attn_mask, qkvo_w):
    global LAST_EXEC_NS
    x = np.ascontiguousarray(np.asarray(x, np.float32))
    ve = np.ascontiguousarray(np.asarray(ve, np.float32))
    sa_lambdas = np.asarray(sa_lambdas, np.float32)
    attn_mask = np.asarray(attn_mask, np.float32)
    qkvo_w = np.asarray(qkvo_w, np.float32)

    ropeC, ropeS = _rope_tables()
    mask = attn_mask[0, 0]
    cls = _classify_blocks(mask)
    has_gen = any(c == "gen" for row in cls for c in row)

    nc = _build_program(cls, has_gen)

    part = np.arange(128)
    tri128 = (part[None, :] >= part[:, None]).astype(ml_dtypes.bfloat16)
    lam0 = np.full((128, 1), sa_lambdas[0], np.float32)
    rCb = np.ascontiguousarray(
        np.tile(ropeC[:, None, :], (1, 2, 1)).reshape(T, 32)
    ).astype(ml_dtypes.bfloat16)
    rSb = np.ascontiguousarray(
        np.tile(ropeS[:, None, :], (1, 2, 1)).reshape(T, 32)
    ).astype(ml_dtypes.bfloat16)
    maskdiv = (mask / SCALE).astype(np.float32) if has_gen else None

    in_maps = []
    for c in range(8):
        b, g = c // G, c % G
        sl = slice(GD * g, GD * (g + 1))
        wqkvT = np.ascontiguousarray(
            np.concatenate([qkvo_w[k][sl, :] for k in range(3)], 0).T)
        m = {
            "xT": np.ascontiguousarray(x[b].T).astype(ml_dtypes.bfloat16),
            "wqkvT": wqkvT.astype(ml_dtypes.bfloat16),
            "woT": np.ascontiguousarray(
                qkvo_w[3][:, sl].T).astype(ml_dtypes.bfloat16),
            "ve_s": np.ascontiguousarray(
                ve[b][:, sl] * sa_lambdas[1]).astype(ml_dtypes.bfloat16),
            "lam0": lam0,
            "ropeC": rCb,
            "ropeS": rSb,
            "tri128": tri128,
        }
        if has_gen:
            m["maskdiv"] = maskdiv
        in_maps.append(m)

    res = run_bass_kernel_spmd(nc, in_maps, core_ids=list(range(8)),
                               trace=TRACE)
    if TRACE:
        LAST_EXEC_NS = res.exec_time_ns

    out = np.zeros((B, T, DIM), np.float32)
    for c in range(8):
        out[c // G] += res.results[c]["outT"].T.astype(np.float32)
    return out
